# revision 2
# baseline (speedup 1.0000x reference)
"""AdaptiveConv2DMod kernel for 8 TRN2 NeuronCores.

Data-parallel over batch: B=16 -> 2 samples per core, base weights replicated.
Per sample: softmax-mix 4 base kernels, modulate by (1+mod) over input
channels, demodulate per output channel, then 3x3 same-conv.

Conv is computed as 9 shifted matmuls (x2 input-channel chunks) accumulated
in PSUM, bf16 compute / fp32 accumulate.
"""

from contextlib import ExitStack

import numpy as np

import concourse.bass as bass
import concourse.mybir as mybir
import concourse.tile as tile
from concourse import bacc
from concourse.bass_utils import run_bass_kernel_spmd

F32 = mybir.dt.float32
BF16 = mybir.dt.bfloat16

N_CORES = 8
B_LOC = 2          # samples per core
C = 256            # input channels (I)
O = 256            # output channels
H = W = 64
K = 3
NK = 4             # num base kernels
CI = 2             # input channel chunks of 128
CO = 2             # output channel chunks of 128
NT = 8             # row tiles (8 rows x 64 cols = 512 free)
ROWS_PER_NT = H // NT
WP = W + 2         # column-padded width


def _build_nc(repeat=1, loop_n=0, parts="full"):
    nc = bacc.Bacc("TRN2", target_bir_lowering=False, debug=False,
                   num_devices=N_CORES)
    fmap = nc.declare_dram_parameter("fmap", [B_LOC, C, H, W], F32, isOutput=False)
    mod = nc.declare_dram_parameter("mod", [B_LOC, C], F32, isOutput=False)
    kmod = nc.declare_dram_parameter("kernel_mod", [B_LOC, NK], F32, isOutput=False)
    weights = nc.declare_dram_parameter("weights", [NK, O, C, K, K], F32,
                                        isOutput=False)
    out = nc.declare_dram_parameter("out", [B_LOC, O, H, W], F32, isOutput=True)

    with ExitStack() as ctx:
        tc = ctx.enter_context(tile.TileContext(nc))
        pools = _make_pools(ctx, tc)
        if loop_n:
            with tc.For_i(0, loop_n, 1):
                _build_body(tc, pools, fmap.ap(), mod.ap(), kmod.ap(),
                            weights.ap(), out.ap(), parts)
        else:
            for _ in range(repeat):
                _build_body(tc, pools, fmap.ap(), mod.ap(), kmod.ap(),
                            weights.ap(), out.ap(), parts)
    _dedupe_ldweights(nc)
    nc.compile()
    return nc


def _dedupe_ldweights(nc):
    """Remove PE weight reloads that are byte-identical to the previous
    Ldweights and carry no semaphore waits/updates (the split emits one
    Ldweights per matmul even when the stationary operand is unchanged)."""
    removed = 0
    pe = mybir.EngineType.PE
    for blk in nc.main_func.blocks:
        last_key = None
        keep = []
        for inst in blk.instructions:
            tn = type(inst).__name__
            eng = getattr(inst, "engine", None)
            if tn == "InstLdweights":
                key = repr(inst.ins)
                if (key == last_key and inst.sync_info is None):
                    removed += 1
                    continue
                last_key = key
            elif tn == "InstMatmult":
                pass
            elif eng == pe:
                last_key = None
            keep.append(inst)
        blk.instructions[:] = keep
    return removed


def _make_pools(ctx, tc):
    return {
        "const": ctx.enter_context(tc.tile_pool(name="const", bufs=2)),
        "wnat": ctx.enter_context(tc.tile_pool(name="wnat", bufs=NK * CO)),
        "mix": ctx.enter_context(tc.tile_pool(name="mix", bufs=4)),
        "wt": ctx.enter_context(tc.tile_pool(name="wt", bufs=B_LOC * CO)),
        "fm": ctx.enter_context(tc.tile_pool(name="fm", bufs=4)),
        "fmraw": ctx.enter_context(tc.tile_pool(name="fmraw", bufs=2)),
        "outp": ctx.enter_context(tc.tile_pool(name="outp", bufs=16)),
        "small": ctx.enter_context(tc.tile_pool(name="small", bufs=10)),
        "psconv": ctx.enter_context(
            tc.tile_pool(name="psconv", bufs=8, space="PSUM")),
    }


def _build_body(tc, pools, fmap, mod, kmod, weights, out, parts="full"):
    nc = tc.nc

    const = pools["const"]
    wnatp = pools["wnat"]
    mixp = pools["mix"]
    wtp = pools["wt"]
    fmp = pools["fm"]
    fmrawp = pools["fmraw"]
    outp = pools["outp"]
    smallp = pools["small"]
    psconv = pools["psconv"]

    # ---- small inputs, broadcast across partitions -------------------------
    kmod_bc = const.tile([128, B_LOC, NK], F32)
    nc.gpsimd.dma_start(out=kmod_bc[:], in_=kmod[None, :, :].broadcast_to(
        [128, B_LOC, NK]))
    m_bc = const.tile([128, B_LOC, C], F32)
    nc.gpsimd.dma_start(out=m_bc[:], in_=mod[None, :, :].broadcast_to(
        [128, B_LOC, C]))
    nc.vector.tensor_scalar_add(m_bc[:], m_bc[:], 1.0)  # 1 + mod

    eps = const.tile([128, 1], F32)
    nc.vector.memset(eps[:], 1e-8)

    # softmax over NK (no max-subtraction; inputs are ~N(0,1))
    esum = const.tile([128, B_LOC], F32)
    attn = const.tile([128, B_LOC, NK], F32)
    nc.scalar.activation(attn[:], kmod_bc[:], mybir.ActivationFunctionType.Exp)
    nc.vector.reduce_sum(esum[:], attn[:], mybir.AxisListType.X)
    nc.vector.reciprocal(esum[:], esum[:])
    for b in range(B_LOC):
        nc.vector.tensor_scalar_mul(attn[:, b, :], attn[:, b, :], esum[:, b:b + 1])

    # ---- input DMAs ---------------------------------------------------------
    # all inputs via SWDGE fp32->bf16 cast; program order = arrival order:
    # weights co0 first (longest pole to the first matmul), then fmap b0
    w_nat = [[None] * CO for _ in range(NK)]
    fm_raw = [[None] * CI for _ in range(B_LOC)]

    def load_weights(co, only_ci=None):
        if co == 0:
            # split the startup-critical chunk per ci half (4.6KB DRAM runs
            # stay contiguous) so the first mix starts after ~2.4MB, not 4.7
            for ci in range(CI):
                if only_ci is not None and ci != only_ci:
                    continue
                for n in range(NK):
                    t = wnatp.tile([128, 128, K * K], BF16, tag="wnat",
                                   bufs=NK * CI, name=f"wnat{n}_{co}_{ci}")
                    nc.gpsimd.dma_start(
                        out=t[:],
                        in_=weights[n, co * 128:(co + 1) * 128,
                                    ci * 128:(ci + 1) * 128, :, :])
                    w_nat[n][co] = w_nat[n][co] or [None] * CI
                    w_nat[n][co][ci] = t
        else:
            for n in range(NK):
                t = wnatp.tile([128, C, K * K], BF16, tag="wnatf",
                               bufs=NK, name=f"wnat{n}_{co}")
                nc.gpsimd.dma_start(
                    out=t[:], in_=weights[n, co * 128:(co + 1) * 128, :, :, :])
                w_nat[n][co] = t

    def wnat_slice(n, co, ci):
        t = w_nat[n][co]
        if isinstance(t, list):
            return t[ci][:]
        return t[:, ci * 128:(ci + 1) * 128, :]

    def load_fmap(b, ci):
        raw = fmrawp.tile([128, H, W], BF16, tag="fmraw",
                          name=f"fmraw{b}_{ci}")
        nc.gpsimd.dma_start(
            out=raw[:], in_=fmap[b, ci * 128:(ci + 1) * 128, :, :])
        fm_raw[b][ci] = raw

    load_weights(0, only_ci=0)
    load_fmap(0, 0)
    load_weights(0, only_ci=1)
    load_fmap(0, 1)
    load_weights(1)
    load_fmap(1, 0)
    load_fmap(1, 1)

    # ---- fmap column-pad + cast to bf16 (DVE) ------------------------------
    fm_cp = [[None] * CI for _ in range(B_LOC)]

    def pad_fmap(b, ci):
        t = fmp.tile([128, H, WP], BF16, tag="fmcp", name=f"fmcp{b}_{ci}")
        nc.vector.memset(t[:, :, 0:1], 0.0)
        nc.vector.memset(t[:, :, WP - 1:WP], 0.0)
        nc.vector.tensor_copy(t[:, :, 1:W + 1], fm_raw[b][ci][:])
        fm_cp[b][ci] = t

    # ---- per-sample weight pipeline ----------------------------------------
    # w_T[b][co]: [128(i in chunk), (ci,kl)=18, o-chunk=128] bf16 modulated
    # transposed weights; one xbar transpose per (b, co, ci) half: column-tile
    # t of the [128, 1152] input maps to t = ci*9 + kl, partition = i % 128.
    w_T = [[None] * CO for _ in range(B_LOC)]
    dscale = [[None] * CO for _ in range(B_LOC)]
    den_h = [[[None] * CI for _ in range(CO)] for _ in range(B_LOC)]

    def weight_pipe(b, co, ci, transposes=True):
        if ci == 0:
            wt = wtp.tile([128, K * K * CI, 128], BF16, tag="wt",
                          name=f"wT{b}_{co}")
            w_T[b][co] = wt
            if not transposes:
                nc.vector.memset(wt[:], 0.25)
        wt = w_T[b][co]
        wn = [wnat_slice(n, co, ci) for n in range(NK)]
        t0 = mixp.tile([128, 128, K * K], BF16, tag="mixa")
        t1 = mixp.tile([128, 128, K * K], BF16, tag="mixb")
        nc.vector.tensor_scalar_mul(t0[:], wn[0], attn[:, b, 0:1])
        nc.vector.tensor_scalar_mul(t1[:], wn[1], attn[:, b, 1:2])
        nc.vector.tensor_add(t0[:], t0[:], t1[:])
        nc.vector.tensor_scalar_mul(t1[:], wn[2], attn[:, b, 2:3])
        nc.vector.tensor_add(t0[:], t0[:], t1[:])
        nc.vector.tensor_scalar_mul(t1[:], wn[3], attn[:, b, 3:4])
        nc.vector.tensor_add(t0[:], t0[:], t1[:])
        # modulate: w *= (1 + mod[i]); write in (kl, i) order so the tap
        # slices are contiguous for the xbar transpose below
        wmod = mixp.tile([128, K * K, 128], BF16, tag="wmod")
        nc.vector.tensor_mul(
            wmod.rearrange("p kl c -> p c kl"), t0[:],
            m_bc[:, b, ci * 128:(ci + 1) * 128, None].broadcast_to(
                [128, 128, K * K]))
        # demod denominator half: sum over free dims of wmod^2 (per o-part)
        sqscratch = mixp.tile([128, K * K, 128], BF16, tag="sqs", bufs=1)
        dh = smallp.tile([128, 1], F32, tag="den", name=f"den{b}_{co}_{ci}")
        nc.scalar.activation(
            sqscratch[:], wmod[:],
            mybir.ActivationFunctionType.Square, accum_out=dh[:])
        den_h[b][co][ci] = dh
        if transposes:
            nc.sync.dma_start(out=wt[:, ci * K * K:(ci + 1) * K * K, :],
                              in_=wmod[:], transpose=True)

    def finish_dscale(b, co):
        ds = smallp.tile([128, 1], F32, tag="dsc")
        nc.vector.tensor_add(ds[:], den_h[b][co][0][:], den_h[b][co][1][:])
        nc.scalar.activation(ds[:], ds[:],
                             mybir.ActivationFunctionType.Sqrt, bias=eps[:])
        nc.vector.reciprocal(ds[:], ds[:])
        dscale[b][co] = ds

    def pipes(b, transposes=True):
        for co in range(CO):
            for ci in range(CI):
                weight_pipe(b, co, ci, transposes)
                if co == 0:
                    pad_fmap(b, ci)
            finish_dscale(b, co)

    if parts == "wdma":
        for b in range(B_LOC):
            for ci in range(CI):
                pad_fmap(b, ci)
    if parts not in ("conv", "wdma"):
        pipes(0, transposes=(parts != "wnotr"))
        pipes(1, transposes=(parts != "wnotr"))
    if parts == "conv":
        for b in range(B_LOC):
            for ci in range(CI):
                pad_fmap(b, ci)
            for co in range(CO):
                wt = wtp.tile([128, K * K * CI, 128], BF16, tag="wt",
                              name=f"wTd{b}_{co}")
                nc.vector.memset(wt[:], 0.25)
                w_T[b][co] = wt
                ds = smallp.tile([128, 1], F32, tag="dsc")
                nc.vector.memset(ds[:], 1.0)
                dscale[b][co] = ds

    # ---- conv: out[o, y, x] += sum_{ci,ky,kx} w.T @ fmap_shifted -----------
    def conv(b, co):
        # ci-outer: all ci0 taps stream while ci1's weights/fmap still load;
        # psum groups stay open across both ci passes
        ps = [psconv.tile([128, ROWS_PER_NT * W], F32, tag="ps",
                          name=f"ps{b}_{co}_{nt}")
              for nt in range(NT)]
        # tap-outer / nt-inner: the stationary lhsT is constant across the 8
        # row-tiles, so _dedupe_ldweights drops 7/8 of the PE weight loads
        for ci in range(CI):
            for ky in range(K):
                for kx in range(K):
                    kl = ky * K + kx
                    lhsT = w_T[b][co][:, ci * K * K + kl, :]
                    for nt in range(NT):
                        y0 = nt * ROWS_PER_NT
                        r0 = y0 + ky - 1          # first input row
                        ny = ROWS_PER_NT
                        psoff = 0
                        if r0 < 0:                # clamp top (ky=0, nt=0)
                            r0, ny, psoff = 0, ROWS_PER_NT - 1, W
                        if r0 + ny > H:           # clamp bottom
                            ny = H - r0
                        rhs = fm_cp[b][ci][:, r0:r0 + ny, kx:kx + W]
                        nc.tensor.matmul(
                            ps[nt][:, psoff:psoff + ny * W],
                            lhsT, rhs,
                            start=(ci == 0 and kl == 0),
                            stop=(ci == CI - 1 and kl == K * K - 1))
        for nt in range(NT):
            ot = outp.tile([128, ROWS_PER_NT * W], F32, tag="ot")
            nc.scalar.mul(ot[:], ps[nt][:], dscale[b][co][:])
            nc.scalar.dma_start(
                out=out[b, co * 128:(co + 1) * 128,
                        nt * ROWS_PER_NT:(nt + 1) * ROWS_PER_NT, :],
                in_=ot[:])

    if parts not in ("wpipe", "wdma", "wnotr"):
        for b in range(B_LOC):
            for co in range(CO):
                conv(b, co)


_NC_CACHE = {}


def _get_nc(repeat=1, loop_n=0, parts="full"):
    key = (repeat, loop_n, parts)
    if key not in _NC_CACHE:
        _NC_CACHE[key] = _build_nc(repeat, loop_n, parts)
    return _NC_CACHE[key]


def _make_in_maps(fmap, mod, kernel_mod, weights):
    in_maps = []
    for c in range(N_CORES):
        s = slice(c * B_LOC, (c + 1) * B_LOC)
        in_maps.append({
            "fmap": np.ascontiguousarray(fmap[s]),
            "mod": np.ascontiguousarray(mod[s]),
            "kernel_mod": np.ascontiguousarray(kernel_mod[s]),
            "weights": weights,
        })
    return in_maps


def kernel(fmap, mod, kernel_mod, weights, _trace=False):
    fmap = np.asarray(fmap, dtype=np.float32)
    mod = np.asarray(mod, dtype=np.float32)
    kernel_mod = np.asarray(kernel_mod, dtype=np.float32)
    weights = np.ascontiguousarray(np.asarray(weights, dtype=np.float32))

    nc = _get_nc()
    in_maps = _make_in_maps(fmap, mod, kernel_mod, weights)
    res = run_bass_kernel_spmd(nc, in_maps, list(range(N_CORES)), trace=_trace)
    outs = np.concatenate([res.results[c]["out"] for c in range(N_CORES)], axis=0)
    if _trace:
        kernel.last_results = res
    return outs



# revision 27
# speedup vs baseline: 1.3052x; 1.3052x over previous
"""AdaptiveConv2DMod kernel for 8 TRN2 NeuronCores.

Data-parallel over batch: B=16 -> 2 samples per core, base weights replicated.
Per sample: softmax-mix 4 base kernels, modulate by (1+mod) over input
channels, demodulate per output channel, then 3x3 same-conv.

Conv is computed as 9 shifted matmuls (x2 input-channel chunks) accumulated
in PSUM, bf16 compute / fp32 accumulate. fmap/weights are cast to bf16 on
the host (the kernel computed in bf16 anyway) to halve input DMA; output is
written bf16 and upcast on the host.

Row-tile groups of 4: within a group the taps are outer and the row tiles
inner, so the stationary PE weights are reused 4x (deduped Ldweights) while
group drains still overlap the next group's matmuls.
"""

from contextlib import ExitStack

import numpy as np

import concourse.bass as bass
import concourse.mybir as mybir
import concourse.tile as tile
from concourse import bacc
from concourse.bass_utils import run_bass_kernel_spmd

F32 = mybir.dt.float32
BF16 = mybir.dt.bfloat16
NP_BF16 = mybir.dt.np(BF16)

N_CORES = 8
B_LOC = 2          # samples per core
C = 256            # input channels (I)
O = 256            # output channels
H = W = 64
K = 3
NK = 4             # num base kernels
CI = 2             # input channel chunks of 128
CO = 2             # output channel chunks of 128
NT = 8             # row tiles (8 rows x 64 cols = 512 free)
GRP = 4            # row tiles per drain group
ROWS_PER_NT = H // NT


def _build_nc(repeat=1, loop_n=0, parts="full"):
    nc = bacc.Bacc("TRN2", target_bir_lowering=False, debug=False,
                   num_devices=N_CORES)
    fmap = nc.declare_dram_parameter("fmap", [B_LOC, C, H, W], BF16,
                                     isOutput=False)
    mod = nc.declare_dram_parameter("mod", [B_LOC, C], F32, isOutput=False)
    kmod = nc.declare_dram_parameter("kernel_mod", [B_LOC, NK], F32,
                                     isOutput=False)
    # weights are host-permuted to [n, o, ci, ky, kx, i128] so the on-chip
    # mix pipeline is (kl, i)-ordered with packed last dims throughout (DVE
    # 2x mode), the per-tap transpose input needs no strided rearrange, and
    # each per-(n, co, ci) tile is one DMA of contiguous 2.3KB runs
    weights = nc.declare_dram_parameter("weights", [NK, O, CI, K, K, 128],
                                        BF16, isOutput=False)
    out = nc.declare_dram_parameter("out", [B_LOC, O, H, W], BF16,
                                    isOutput=True)

    with ExitStack() as ctx:
        tc = ctx.enter_context(tile.TileContext(nc))
        pools = _make_pools(ctx, tc)
        if loop_n:
            with tc.For_i(0, loop_n, 1):
                _build_body(tc, pools, fmap.ap(), mod.ap(), kmod.ap(),
                            weights.ap(), out.ap(), parts)
        else:
            for _ in range(repeat):
                _build_body(tc, pools, fmap.ap(), mod.ap(), kmod.ap(),
                            weights.ap(), out.ap(), parts)
    _dedupe_ldweights(nc)
    nc.compile()
    return nc


def _dedupe_ldweights(nc):
    """Remove PE weight reloads that are byte-identical to the previous
    Ldweights and carry no semaphore waits/updates (the split emits one
    Ldweights per matmul even when the stationary operand is unchanged)."""
    removed = 0
    pe = mybir.EngineType.PE
    for blk in nc.main_func.blocks:
        last_key = None
        keep = []
        for inst in blk.instructions:
            tn = type(inst).__name__
            eng = getattr(inst, "engine", None)
            if tn == "InstLdweights":
                key = repr(inst.ins)
                if (key == last_key and inst.sync_info is None):
                    removed += 1
                    continue
                last_key = key
            elif tn == "InstMatmult":
                pass
            elif eng == pe:
                last_key = None
            keep.append(inst)
        blk.instructions[:] = keep
    return removed


def _make_pools(ctx, tc):
    return {
        "const": ctx.enter_context(tc.tile_pool(name="const", bufs=2)),
        "wnat": ctx.enter_context(tc.tile_pool(name="wnat", bufs=3)),
        "mix": ctx.enter_context(tc.tile_pool(name="mix", bufs=4)),
        "wt": ctx.enter_context(tc.tile_pool(name="wt", bufs=B_LOC * CO)),
        "fm": ctx.enter_context(tc.tile_pool(name="fm", bufs=4)),
        "outp": ctx.enter_context(tc.tile_pool(name="outp", bufs=4)),
        "small": ctx.enter_context(tc.tile_pool(name="small", bufs=12)),
        "psconv": ctx.enter_context(
            tc.tile_pool(name="psconv", bufs=8, space="PSUM")),
    }


def _build_body(tc, pools, fmap, mod, kmod, weights, out, parts="full"):
    nc = tc.nc

    const = pools["const"]
    wnatp = pools["wnat"]
    mixp = pools["mix"]
    wtp = pools["wt"]
    fmp = pools["fm"]
    outp = pools["outp"]
    smallp = pools["small"]
    psconv = pools["psconv"]

    # ---- small inputs, broadcast across partitions (sync queue) ------------
    kmod_bc = const.tile([128, B_LOC, NK], F32)
    nc.sync.dma_start(out=kmod_bc[:], in_=kmod[None, :, :].broadcast_to(
        [128, B_LOC, NK]))
    # mod lands as one tiny single-partition DMA; broadcast on-chip (the
    # 128-partition broadcast DMA would sit on the startup-critical DMA path)
    m_sm = const.tile([1, B_LOC, C], F32)
    nc.sync.dma_start(out=m_sm[:], in_=mod[None, :, :])
    m_bc = const.tile([128, B_LOC, C], F32)

    eps = const.tile([128, 1], F32)
    nc.vector.memset(eps[:], 1e-8)

    # softmax over NK (no max-subtraction; inputs are ~N(0,1)); emitted
    # before the m_bc add so the in-order DVE queue isn't blocked on the
    # m_bc DMA before computing attn (which gates the first weight mix)
    esum = const.tile([128, B_LOC], F32)
    attn = const.tile([128, B_LOC, NK], F32)
    nc.scalar.activation(attn[:], kmod_bc[:], mybir.ActivationFunctionType.Exp)
    # preload the Square/Sqrt activation tables off the critical path (the
    # auto-inserted LoadActFuncSet otherwise serializes before the first
    # demod Square)
    preld = const.tile([128, 1], F32)
    nc.scalar.activation(preld[:], eps[:], mybir.ActivationFunctionType.Square)
    nc.scalar.activation(preld[:], eps[:], mybir.ActivationFunctionType.Sqrt,
                         bias=eps[:])
    nc.vector.reduce_sum(esum[:], attn[:], mybir.AxisListType.X)
    nc.vector.reciprocal(esum[:], esum[:])
    for b in range(B_LOC):
        nc.vector.tensor_scalar_mul(attn[:, b, :], attn[:, b, :], esum[:, b:b + 1])
    # the (1 + mod) add is emitted lazily just before its first use so the
    # in-order DVE queue isn't blocked on the m_bc DMA before the first mix
    m_ready = []

    def ensure_m():
        if not m_ready:
            nc.gpsimd.partition_broadcast(m_bc[:], m_sm[0:1, :, :])
            nc.vector.tensor_scalar_add(m_bc[:], m_bc[:], 1.0)  # 1 + mod
            m_ready.append(True)

    # ---- input DMAs: all on the Pool queue, in first-use priority order ----
    # (single queue => transfer order == issue order, so fmaps can't steal
    # DMA bandwidth from the startup-critical weight chunks)
    # w9[co][ci]: [128(o), NK, 9(kl), 128(i)] bf16, one DMA each with
    # contiguous 2.3KB runs
    w9 = [[None] * CI for _ in range(CO)]
    fm_raw = [[None] * CI for _ in range(B_LOC)]

    def load_w(co, ci):
        t = wnatp.tile([128, NK, K * K, 128], BF16, tag=f"wn{co}{ci}",
                       bufs=1, name=f"w9_{co}_{ci}")
        nc.gpsimd.dma_start(
            out=t[:],
            in_=weights[:, co * 128:(co + 1) * 128, ci, :, :, :].rearrange(
                "n o ky kx i -> o n (ky kx) i"))
        w9[co][ci] = t

    def w9_slice(n, co, ci):
        return w9[co][ci][:, n, :, :]

    def load_fmap(b, ci):
        raw = fmp.tile([128, H, W], BF16, tag="fmraw", name=f"fmraw{b}_{ci}")
        nc.gpsimd.dma_start(
            out=raw[:], in_=fmap[b, ci * 128:(ci + 1) * 128, :, :])
        fm_raw[b][ci] = raw

    # NOTE: load emission is interleaved with the weight-pipe blocks below —
    # Tile chains each DMA-family instruction to the completion of the one
    # emitted just before it, so a transpose must not be preceded by a DMA
    # it doesn't actually need.

    # ---- per-sample weight pipeline ----------------------------------------
    # w_T[b][co]: [128(i in chunk), (ci,kl)=18, o-chunk=128] bf16 modulated
    # transposed weights; one xbar transpose per (b, co, ci) half.
    w_T = [[None] * CO for _ in range(B_LOC)]
    dscale = [[None] * CO for _ in range(B_LOC)]
    den_h = [[[None] * CI for _ in range(CO)] for _ in range(B_LOC)]

    def weight_pipe(b, co, ci, transposes=True):
        if ci == 0:
            wt = wtp.tile([128, K * K * CI, 128], BF16, tag="wt",
                          name=f"wT{b}_{co}")
            w_T[b][co] = wt
            if not transposes:
                nc.vector.memset(wt[:], 0.25)
        wt = w_T[b][co]
        wn = [w9_slice(n, co, ci) for n in range(NK)]
        t0 = mixp.tile([128, K * K, 128], BF16, tag="mixa")
        t1 = mixp.tile([128, K * K, 128], BF16, tag="mixb")
        t2 = mixp.tile([128, K * K, 128], BF16, tag="mixc")
        t3 = mixp.tile([128, K * K, 128], BF16, tag="mixd")
        # n2/n3 scaled on ACT to shorten the DVE latency chain
        nc.scalar.mul(t2[:], wn[2], attn[:, b, 2:3])
        nc.scalar.mul(t3[:], wn[3], attn[:, b, 3:4])
        nc.vector.tensor_scalar_mul(t0[:], wn[0], attn[:, b, 0:1])
        nc.vector.tensor_scalar_mul(t1[:], wn[1], attn[:, b, 1:2])
        nc.vector.tensor_add(t0[:], t0[:], t1[:])
        nc.vector.tensor_add(t2[:], t2[:], t3[:])
        nc.vector.tensor_add(t0[:], t0[:], t2[:])
        # modulate: w *= (1 + mod[i]); everything is (kl, i)-ordered so all
        # operands stay packed and the transpose input needs no rearrange
        ensure_m()
        wmod = mixp.tile([128, K * K, 128], BF16, tag="wmod")
        nc.vector.tensor_mul(
            wmod[:], t0[:],
            m_bc[:, b, None, ci * 128:(ci + 1) * 128].broadcast_to(
                [128, K * K, 128]))
        # transpose first (it gates the conv), then the demod denominator
        # half: sum over free dims of wmod^2 (per o-part)
        if transposes:
            nc.sync.dma_start(out=wt[:, ci * K * K:(ci + 1) * K * K, :],
                              in_=wmod[:], transpose=True)
        sqscratch = mixp.tile([128, K * K, 128], BF16, tag="sqs", bufs=2)
        dh = smallp.tile([128, 1], F32, tag="den", name=f"den{b}_{co}_{ci}")
        nc.scalar.activation(
            sqscratch[:], wmod[:],
            mybir.ActivationFunctionType.Square, accum_out=dh[:])
        den_h[b][co][ci] = dh

    def finish_dscale(b, co):
        ds = smallp.tile([128, 1], F32, tag="dsc")
        nc.vector.tensor_add(ds[:], den_h[b][co][0][:], den_h[b][co][1][:])
        nc.scalar.activation(ds[:], ds[:],
                             mybir.ActivationFunctionType.Sqrt, bias=eps[:])
        nc.vector.reciprocal(ds[:], ds[:])
        dscale[b][co] = ds

    if parts == "conv":
        for b in range(B_LOC):
            for ci in range(CI):
                load_fmap(b, ci)
            for co in range(CO):
                wt = wtp.tile([128, K * K * CI, 128], BF16, tag="wt",
                              name=f"wTd{b}_{co}")
                nc.vector.memset(wt[:], 0.25)
                w_T[b][co] = wt
                ds = smallp.tile([128, 1], F32, tag="dsc")
                nc.vector.memset(ds[:], 1.0)
                dscale[b][co] = ds
    else:
        tr = parts != "wnotr"
        load_w(0, 0)
        load_fmap(0, 0)
        weight_pipe(0, 0, 0, tr)
        load_w(0, 1)
        load_fmap(0, 1)
        weight_pipe(0, 0, 1, tr)
        finish_dscale(0, 0)
        load_w(1, 0)
        load_w(1, 1)
        weight_pipe(0, 1, 0, tr)
        weight_pipe(0, 1, 1, tr)
        finish_dscale(0, 1)
        load_fmap(1, 0)
        load_fmap(1, 1)
        for co in range(CO):
            for ci in range(CI):
                weight_pipe(1, co, ci, tr)
            finish_dscale(1, co)

    # ---- conv: out[o, y, x] += sum_{ci,ky,kx} w.T @ fmap_shifted -----------
    # Row-tile groups of GRP. Within a group: ci -> tap -> row-tile, so the
    # stationary weights are constant across the GRP inner matmuls (deduped
    # Ldweights) and each group's psum drains overlap the next group.
    # Boundary taps use column/row-clamped access patterns; elements a tap
    # skips are covered by other taps' writes (per-element has_written).
    def conv(b, co):
        for g in range(NT // GRP):
            nts = range(g * GRP, (g + 1) * GRP)
            ps = {nt: psconv.tile([128, ROWS_PER_NT, W], F32, tag="ps",
                                  name=f"ps{b}_{co}_{nt}")
                  for nt in nts}
            for ci in range(CI):
                for ky in range(K):
                    for kx in range(K):
                        kl = ky * K + kx
                        lhsT = w_T[b][co][:, ci * K * K + kl, :]
                        x0 = max(0, kx - 1)
                        xo = max(0, 1 - kx)
                        wn = W - abs(kx - 1)
                        for nt in nts:
                            r0 = nt * ROWS_PER_NT + ky - 1
                            ny = ROWS_PER_NT
                            rskip = 0
                            if r0 < 0:                # clamp top
                                r0, ny, rskip = 0, ROWS_PER_NT - 1, 1
                            if r0 + ny > H:           # clamp bottom
                                ny = H - r0
                            rhs = fm_raw[b][ci][:, r0:r0 + ny, x0:x0 + wn]
                            nc.tensor.matmul(
                                ps[nt][:, rskip:rskip + ny, xo:xo + wn],
                                lhsT, rhs,
                                start=(ci == 0 and kl == 0),
                                stop=(ci == CI - 1 and kl == K * K - 1))
            # drain group: scale into bf16 (split over ACT and DVE so the
            # final group's tail is short), one batched DMA out
            ot = outp.tile([128, GRP * ROWS_PER_NT, W], BF16, tag="ot")
            for j, nt in enumerate(nts):
                osl = ot[:, j * ROWS_PER_NT:(j + 1) * ROWS_PER_NT, :]
                if j % 2 == 0:
                    nc.scalar.mul(osl, ps[nt][:], dscale[b][co][:])
                else:
                    nc.vector.tensor_scalar_mul(osl, ps[nt][:],
                                                dscale[b][co][:])
            nc.gpsimd.dma_start(
                out=out[b, co * 128:(co + 1) * 128,
                        g * GRP * ROWS_PER_NT:(g + 1) * GRP * ROWS_PER_NT, :],
                in_=ot[:])

    if parts not in ("wpipe", "wnotr"):
        for b in range(B_LOC):
            for co in range(CO):
                conv(b, co)


_NC_CACHE = {}


def _get_nc(repeat=1, loop_n=0, parts="full"):
    key = (repeat, loop_n, parts)
    if key not in _NC_CACHE:
        _NC_CACHE[key] = _build_nc(repeat, loop_n, parts)
    return _NC_CACHE[key]


def _make_in_maps(fmap, mod, kernel_mod, weights):
    fmap_bf = np.ascontiguousarray(fmap.astype(NP_BF16))
    # [n, o, i, ky, kx] -> [n, o, ci, ky, kx, i128] (see DRAM declaration)
    weights_bf = np.ascontiguousarray(
        weights.astype(NP_BF16)
        .reshape(NK, O, CI, 128, K, K)
        .transpose(0, 1, 2, 4, 5, 3))
    in_maps = []
    for c in range(N_CORES):
        s = slice(c * B_LOC, (c + 1) * B_LOC)
        in_maps.append({
            "fmap": np.ascontiguousarray(fmap_bf[s]),
            "mod": np.ascontiguousarray(mod[s]),
            "kernel_mod": np.ascontiguousarray(kernel_mod[s]),
            "weights": weights_bf,
        })
    return in_maps


def kernel(fmap, mod, kernel_mod, weights, _trace=False):
    fmap = np.asarray(fmap, dtype=np.float32)
    mod = np.asarray(mod, dtype=np.float32)
    kernel_mod = np.asarray(kernel_mod, dtype=np.float32)
    weights = np.ascontiguousarray(np.asarray(weights, dtype=np.float32))

    nc = _get_nc()
    in_maps = _make_in_maps(fmap, mod, kernel_mod, weights)
    res = run_bass_kernel_spmd(nc, in_maps, list(range(N_CORES)), trace=_trace)
    outs = np.concatenate(
        [res.results[c]["out"].astype(np.float32) for c in range(N_CORES)],
        axis=0)
    if _trace:
        kernel.last_results = res
    return outs


# revision 39
# speedup vs baseline: 1.7324x; 1.3273x over previous
"""AdaptiveConv2DMod kernel for 8 TRN2 NeuronCores.

Data-parallel over batch: B=16 -> 2 samples per core, base weights replicated.
Per sample: softmax-mix 4 base kernels, modulate by (1+mod) over input
channels, demodulate per output channel, then 3x3 same-conv.

Conv is computed as 9 shifted matmuls (x2 input-channel chunks) accumulated
in PSUM, bf16 compute / fp32 accumulate. fmap/weights are cast to bf16 on
the host (the kernel computed in bf16 anyway) to halve input DMA; softmax over
the 4 kernel logits is computed on the host; output is written fp32 from
the demod-scaling psum drain.

Row-tile groups of 4: within a group the taps are outer and the row tiles
inner, so the stationary PE weights are reused 4x (deduped Ldweights) while
group drains still overlap the next group's matmuls.
"""

from contextlib import ExitStack

import numpy as np

import concourse.bass as bass
import concourse.mybir as mybir
import concourse.tile as tile
from concourse import bacc
from concourse.bass_utils import run_bass_kernel_spmd

F32 = mybir.dt.float32
BF16 = mybir.dt.bfloat16
NP_BF16 = mybir.dt.np(BF16)

N_CORES = 8
B_LOC = 2          # samples per core
C = 256            # input channels (I)
O = 256            # output channels
H = W = 64
K = 3
NK = 4             # num base kernels
CI = 2             # input channel chunks of 128
CO = 2             # output channel chunks of 128
NT = 8             # row tiles (8 rows x 64 cols = 512 free)
GRP = 4            # row tiles per drain group
ROWS_PER_NT = H // NT


def _build_nc(repeat=1, loop_n=0, parts="full"):
    nc = bacc.Bacc("TRN2", target_bir_lowering=False, debug=False,
                   num_devices=N_CORES)
    fmap = nc.declare_dram_parameter("fmap", [B_LOC, C, H, W], BF16,
                                     isOutput=False)
    mod = nc.declare_dram_parameter("mod", [B_LOC, C], F32, isOutput=False)
    # softmax(kernel_mod) is computed on the host (tiny [B,4] op) so the
    # device pipeline has no ACT dependency before the first weight mix
    kmod = nc.declare_dram_parameter("attn_in", [B_LOC, NK], F32,
                                     isOutput=False)
    # weights are host-permuted to [n, o, ci, ky, kx, i128] so the on-chip
    # mix pipeline is (kl, i)-ordered with packed last dims throughout (DVE
    # 2x mode), the per-tap transpose input needs no strided rearrange, and
    # each per-(n, co, ci) tile is one DMA of contiguous 2.3KB runs
    weights = nc.declare_dram_parameter("weights", [NK, O, CI, K, K, 128],
                                        BF16, isOutput=False)
    out = nc.declare_dram_parameter("out", [B_LOC, O, H, W], F32,
                                    isOutput=True)

    with ExitStack() as ctx:
        tc = ctx.enter_context(tile.TileContext(nc))
        pools = _make_pools(ctx, tc)
        if loop_n:
            with tc.For_i(0, loop_n, 1):
                _build_body(tc, pools, fmap.ap(), mod.ap(), kmod.ap(),
                            weights.ap(), out.ap(), parts)
        else:
            for _ in range(repeat):
                _build_body(tc, pools, fmap.ap(), mod.ap(), kmod.ap(),
                            weights.ap(), out.ap(), parts)
    _dedupe_ldweights(nc)
    nc.compile()
    return nc


def _dedupe_ldweights(nc):
    """Remove PE weight reloads that are byte-identical to the previous
    Ldweights and carry no semaphore waits/updates (the split emits one
    Ldweights per matmul even when the stationary operand is unchanged)."""
    removed = 0
    pe = mybir.EngineType.PE
    for blk in nc.main_func.blocks:
        last_key = None
        keep = []
        for inst in blk.instructions:
            tn = type(inst).__name__
            eng = getattr(inst, "engine", None)
            if tn == "InstLdweights":
                key = repr(inst.ins)
                if (key == last_key and inst.sync_info is None):
                    removed += 1
                    continue
                last_key = key
            elif tn == "InstMatmult":
                pass
            elif eng == pe:
                last_key = None
            keep.append(inst)
        blk.instructions[:] = keep
    return removed


def _make_pools(ctx, tc):
    return {
        "const": ctx.enter_context(tc.tile_pool(name="const", bufs=2)),
        "wnat": ctx.enter_context(tc.tile_pool(name="wnat", bufs=3)),
        "mix": ctx.enter_context(tc.tile_pool(name="mix", bufs=4)),
        "wt": ctx.enter_context(tc.tile_pool(name="wt", bufs=B_LOC * CO)),
        "fm": ctx.enter_context(tc.tile_pool(name="fm", bufs=4)),
        "outp": ctx.enter_context(tc.tile_pool(name="outp", bufs=2)),
        "small": ctx.enter_context(tc.tile_pool(name="small", bufs=12)),
        "psconv": ctx.enter_context(
            tc.tile_pool(name="psconv", bufs=8, space="PSUM")),
    }


def _build_body(tc, pools, fmap, mod, kmod, weights, out, parts="full"):
    nc = tc.nc

    const = pools["const"]
    wnatp = pools["wnat"]
    mixp = pools["mix"]
    wtp = pools["wt"]
    fmp = pools["fm"]
    outp = pools["outp"]
    smallp = pools["small"]
    psconv = pools["psconv"]

    # ---- small inputs (sync queue) -----------------------------------------
    attn = const.tile([128, B_LOC, NK], F32)
    nc.sync.dma_start(out=attn[:], in_=kmod[None, :, :].broadcast_to(
        [128, B_LOC, NK]))
    # mod lands as one tiny single-partition DMA; broadcast on-chip (the
    # 128-partition broadcast DMA would sit on the startup-critical DMA path)
    m_sm = const.tile([1, B_LOC, C], F32)
    nc.sync.dma_start(out=m_sm[:], in_=mod[None, :, :])
    m_bc = const.tile([128, B_LOC, C], F32)

    eps = const.tile([128, 1], F32)
    nc.vector.memset(eps[:], 1e-8)

    # preload the Square/Sqrt activation tables off the critical path (the
    # auto-inserted LoadActFuncSet otherwise serializes before the first
    # demod Square)
    preld = const.tile([128, 1], F32)
    nc.scalar.activation(preld[:], eps[:], mybir.ActivationFunctionType.Square)
    nc.scalar.activation(preld[:], eps[:], mybir.ActivationFunctionType.Sqrt,
                         bias=eps[:])
    # the (1 + mod) add is emitted lazily just before its first use so the
    # in-order DVE queue isn't blocked on the m_bc DMA before the first mix
    m_ready = []

    def ensure_m():
        if not m_ready:
            nc.gpsimd.partition_broadcast(m_bc[:], m_sm[0:1, :, :])
            nc.vector.tensor_scalar_add(m_bc[:], m_bc[:], 1.0)  # 1 + mod
            m_ready.append(True)

    # ---- input DMAs: all on the Pool queue, in first-use priority order ----
    # (single queue => transfer order == issue order, so fmaps can't steal
    # DMA bandwidth from the startup-critical weight chunks)
    # w9[co][ci]: [128(o), NK, 9(kl), 128(i)] bf16, one DMA each with
    # contiguous 2.3KB runs
    w9 = [[None] * CI for _ in range(CO)]
    fm_raw = [[None] * CI for _ in range(B_LOC)]

    def load_w(co, ci):
        t = wnatp.tile([128, NK, K * K, 128], BF16, tag=f"wn{co}{ci}",
                       bufs=1, name=f"w9_{co}_{ci}")
        nc.gpsimd.dma_start(
            out=t[:],
            in_=weights[:, co * 128:(co + 1) * 128, ci, :, :, :].rearrange(
                "n o ky kx i -> o n (ky kx) i"))
        w9[co][ci] = t

    def w9_slice(n, co, ci):
        return w9[co][ci][:, n, :, :]

    def load_fmap(b, ci):
        raw = fmp.tile([128, H, W], BF16, tag="fmraw", name=f"fmraw{b}_{ci}")
        nc.gpsimd.dma_start(
            out=raw[:], in_=fmap[b, ci * 128:(ci + 1) * 128, :, :])
        fm_raw[b][ci] = raw

    # NOTE: load emission is interleaved with the weight-pipe blocks below —
    # Tile chains each DMA-family instruction to the completion of the one
    # emitted just before it, so a transpose must not be preceded by a DMA
    # it doesn't actually need.

    # ---- per-sample weight pipeline ----------------------------------------
    # w_T[b][co]: [128(i in chunk), (ci,kl)=18, o-chunk=128] bf16 modulated
    # transposed weights; one xbar transpose per (b, co, ci) half.
    w_T = [[None] * CO for _ in range(B_LOC)]
    dscale = [[None] * CO for _ in range(B_LOC)]
    den_h = [[[None] * CI for _ in range(CO)] for _ in range(B_LOC)]

    wmods = [[[None] * CI for _ in range(CO)] for _ in range(B_LOC)]

    def mix_block(b, co, ci, transposes=True):
        """mix 4 base kernels + modulate by (1+mod) -> wmod (kl, i)-ordered."""
        wn = [w9_slice(n, co, ci) for n in range(NK)]
        t0 = mixp.tile([128, K * K, 128], BF16, tag="mixa")
        t1 = mixp.tile([128, K * K, 128], BF16, tag="mixb")
        nc.vector.tensor_scalar_mul(t0[:], wn[0], attn[:, b, 0:1])
        nc.vector.tensor_scalar_mul(t1[:], wn[1], attn[:, b, 1:2])
        nc.vector.tensor_add(t0[:], t0[:], t1[:])
        nc.vector.tensor_scalar_mul(t1[:], wn[2], attn[:, b, 2:3])
        nc.vector.tensor_add(t0[:], t0[:], t1[:])
        nc.vector.tensor_scalar_mul(t1[:], wn[3], attn[:, b, 3:4])
        nc.vector.tensor_add(t0[:], t0[:], t1[:])
        ensure_m()
        wmod = mixp.tile([128, K * K, 128], BF16, tag="wmod", bufs=4)
        nc.vector.tensor_mul(
            wmod[:], t0[:],
            m_bc[:, b, None, ci * 128:(ci + 1) * 128].broadcast_to(
                [128, K * K, 128]))
        wmods[b][co][ci] = wmod
        # transpose fires as soon as this half's wmod is ready (it gates the
        # conv); the demod scale is applied at psum-drain time instead
        if transposes:
            wt = w_T[b][co]
            nc.sync.dma_start(out=wt[:, ci * K * K:(ci + 1) * K * K, :],
                              in_=wmod[:], transpose=True)
        # demod denominator half: sum over free dims of wmod^2 (per o-part)
        sqscratch = mixp.tile([128, K * K, 128], BF16, tag="sqs", bufs=2)
        dh = smallp.tile([128, 1], F32, tag="den", name=f"den{b}_{co}_{ci}")
        nc.scalar.activation(
            sqscratch[:], wmod[:],
            mybir.ActivationFunctionType.Square, accum_out=dh[:])
        den_h[b][co][ci] = dh

    def finalize_block(b, co):
        ds = smallp.tile([128, 1], F32, tag="dsc")
        nc.vector.tensor_add(ds[:], den_h[b][co][0][:], den_h[b][co][1][:])
        nc.scalar.activation(ds[:], ds[:],
                             mybir.ActivationFunctionType.Sqrt, bias=eps[:])
        nc.vector.reciprocal(ds[:], ds[:])
        dscale[b][co] = ds

    def pipe(b, co, transposes=True):
        wt = wtp.tile([128, K * K * CI, 128], BF16, tag="wt",
                      name=f"wT{b}_{co}")
        w_T[b][co] = wt
        if not transposes:
            nc.vector.memset(wt[:], 0.25)
        for ci in range(CI):
            mix_block(b, co, ci, transposes)
        finalize_block(b, co)

    if parts == "conv":
        for b in range(B_LOC):
            for ci in range(CI):
                load_fmap(b, ci)
            for co in range(CO):
                wt = wtp.tile([128, K * K * CI, 128], BF16, tag="wt",
                              name=f"wTd{b}_{co}")
                nc.vector.memset(wt[:], 0.25)
                w_T[b][co] = wt
                ds = smallp.tile([128, 1], F32, tag="dsc")
                nc.vector.memset(ds[:], 1.0)
                dscale[b][co] = ds
    else:
        tr = parts != "wnotr"
        load_w(0, 0)
        load_fmap(0, 0)
        load_w(0, 1)
        pipe(0, 0, tr)
        load_fmap(0, 1)
        load_w(1, 0)
        load_w(1, 1)
        pipe(0, 1, tr)
        load_fmap(1, 0)
        load_fmap(1, 1)
        pipe(1, 0, tr)
        pipe(1, 1, tr)

    # ---- conv: out[o, y, x] += sum_{ci,ky,kx} w.T @ fmap_shifted -----------
    # Row-tile groups of GRP. Within a group: ci -> tap -> row-tile, so the
    # stationary weights are constant across the GRP inner matmuls (deduped
    # Ldweights) and each group's psum drains overlap the next group.
    # Boundary taps use column/row-clamped access patterns; elements a tap
    # skips are covered by other taps' writes (per-element has_written).
    def conv(b, co):
        for g in range(NT // GRP):
            # one 4-bank psum tile per group; each matmul writes a 1-bank
            # slice; the whole group DMAs straight to DRAM (weights carry
            # the demod scale already)
            ps = psconv.tile([128, GRP, ROWS_PER_NT, W], F32, tag="psg",
                             bufs=2, name=f"ps{b}_{co}_{g}")
            for ci in range(CI):
                for ky in range(K):
                    for kx in range(K):
                        kl = ky * K + kx
                        lhsT = w_T[b][co][:, ci * K * K + kl, :]
                        x0 = max(0, kx - 1)
                        xo = max(0, 1 - kx)
                        wn = W - abs(kx - 1)
                        for j in range(GRP):
                            nt = g * GRP + j
                            r0 = nt * ROWS_PER_NT + ky - 1
                            ny = ROWS_PER_NT
                            rskip = 0
                            if r0 < 0:                # clamp top
                                r0, ny, rskip = 0, ROWS_PER_NT - 1, 1
                            if r0 + ny > H:           # clamp bottom
                                ny = H - r0
                            rhs = fm_raw[b][ci][:, r0:r0 + ny, x0:x0 + wn]
                            nc.tensor.matmul(
                                ps[:, j, rskip:rskip + ny, xo:xo + wn],
                                lhsT, rhs,
                                start=(ci == 0 and kl == 0),
                                stop=(ci == CI - 1 and kl == K * K - 1))
            # drain: demod scale applied here (per-o-partition), fp32 out via
            # the ACT HWDGE queue so the Pool queue stays free for input DMAs
            ot = outp.tile([128, GRP * ROWS_PER_NT, W], F32, tag="ot", bufs=2)
            nc.scalar.mul(ot[:], ps.rearrange("p g r w -> p (g r) w"),
                          dscale[b][co][:])
            nc.scalar.dma_start(
                out=out[b, co * 128:(co + 1) * 128,
                        g * GRP * ROWS_PER_NT:(g + 1) * GRP * ROWS_PER_NT, :],
                in_=ot[:])

    if parts not in ("wpipe", "wnotr"):
        for b in range(B_LOC):
            for co in range(CO):
                conv(b, co)


_NC_CACHE = {}


def _get_nc(repeat=1, loop_n=0, parts="full"):
    key = (repeat, loop_n, parts)
    if key not in _NC_CACHE:
        _NC_CACHE[key] = _build_nc(repeat, loop_n, parts)
    return _NC_CACHE[key]


def _make_in_maps(fmap, mod, kernel_mod, weights):
    fmap_bf = np.ascontiguousarray(fmap.astype(NP_BF16))
    # [n, o, i, ky, kx] -> [n, o, ci, ky, kx, i128] (see DRAM declaration)
    weights_bf = np.ascontiguousarray(
        weights.astype(NP_BF16)
        .reshape(NK, O, CI, 128, K, K)
        .transpose(0, 1, 2, 4, 5, 3))
    # host-side softmax over the 4 kernel logits (tiny)
    e = np.exp(kernel_mod.astype(np.float64)
               - kernel_mod.max(axis=-1, keepdims=True))
    attn = (e / e.sum(axis=-1, keepdims=True)).astype(np.float32)
    in_maps = []
    for c in range(N_CORES):
        s = slice(c * B_LOC, (c + 1) * B_LOC)
        in_maps.append({
            "fmap": np.ascontiguousarray(fmap_bf[s]),
            "mod": np.ascontiguousarray(mod[s]),
            "attn_in": np.ascontiguousarray(attn[s]),
            "weights": weights_bf,
        })
    return in_maps


def kernel(fmap, mod, kernel_mod, weights, _trace=False):
    fmap = np.asarray(fmap, dtype=np.float32)
    mod = np.asarray(mod, dtype=np.float32)
    kernel_mod = np.asarray(kernel_mod, dtype=np.float32)
    weights = np.ascontiguousarray(np.asarray(weights, dtype=np.float32))

    nc = _get_nc()
    in_maps = _make_in_maps(fmap, mod, kernel_mod, weights)
    res = run_bass_kernel_spmd(nc, in_maps, list(range(N_CORES)), trace=_trace)
    outs = np.concatenate(
        [res.results[c]["out"].astype(np.float32) for c in range(N_CORES)],
        axis=0)
    if _trace:
        kernel.last_results = res
    return outs
